# revision 40
# baseline (speedup 1.0000x reference)
"""LMU kernel for Trainium2, 8-core data-parallel.

Math (per batch b, with x[b] in [D, L] layout):
  u[b]    = relu(W_u @ x[b] + b_u)                              [1, L]
  m[b]    = H @ Toep(u[b])        (causal conv via Toeplitz)    [D, L]
  h[b]    = relu(W_h[:, :D] @ m[b] + W_h[:, D:] @ x[b] + b_h)   [D, L]
  y[b]    = BN(conv_w @ h[b] + conv_b)                          [D, L]

Device-side folds (host precomputes, O(params) only):
  F      = (W_h[:, :D] @ H).T, row-flipped  -> single K=128 contraction
           against the (flipped) Toeplitz of u
  C'     = (inv * conv_w).T, bias' = (conv_b - mean) * inv + beta   (BN fold)

All matmul operands are bf16 (host-cast), so LDWEIGHTS hides fully under
the 1 col/cycle stream and no on-device casts are needed anywhere.
Batch dim sharded 8 ways; params replicated.
"""

import os
import numpy as np
import ml_dtypes

import concourse.bass as bass
import concourse.mybir as mybir
from concourse import bacc
from concourse.tile import TileContext
from concourse.bass_utils import run_bass_kernel_spmd

B, D, L = 256, 768, 128
NCORES = 8
BPC = B // NCORES          # batches per core
NB = 4                     # batches per column block
NCB = BPC // NB            # column blocks per core
NCOL = NB * L              # 512 columns per block
KC = D // 128              # 6 chunks of 128 over the D dim
THETA = 128.0
BN_EPS = 1e-5

TRACE = False
LAST_EXEC_NS = None

_H_CACHE = None
_NC_CACHE = None


def _impulse_response():
    """Replicates the reference's H = impulse response [D, L], on CPU."""
    global _H_CACHE
    if _H_CACHE is not None:
        return _H_CACHE
    import jax
    import jax.numpy as jnp
    from jax.scipy.linalg import expm

    cpu = jax.devices("cpu")[0]
    with jax.default_device(cpu):
        Q = np.arange(D, dtype=np.float32)
        R = ((2.0 * Q + 1.0) / THETA)[:, None]
        i, j = np.meshgrid(Q, Q, indexing="ij")
        A = (np.where(i < j, -1.0, (-1.0) ** (i - j + 1)).astype(np.float32)) * R
        Bm = (((-1.0) ** Q)[:, None]).astype(np.float32) * R
        Maug = np.zeros((D + 1, D + 1), dtype=np.float32)
        Maug[:D, :D] = A
        Maug[:D, D:] = Bm
        E = expm(jnp.asarray(Maug))
        Ad = E[:D, :D]
        Bd = E[:D, D:]

        def step(Apow, _):
            return Ad @ Apow, (Apow @ Bd)[:, 0]

        _, H = jax.lax.scan(step, jnp.eye(D, dtype=jnp.float32), None, length=L)
        _H_CACHE = np.asarray(H).T.astype(np.float32)  # [D, L]
    return _H_CACHE


def _build_nc():
    """Builds the (static) 8-core SPMD Bass program."""
    f32 = mybir.dt.float32
    bf16 = mybir.dt.bfloat16
    nc = bacc.Bacc("TRN2", target_bir_lowering=False, debug=False, num_devices=NCORES)

    x_d = nc.dram_tensor("x", [D, BPC, L], bf16, kind="ExternalInput").ap()
    whxT_d = nc.dram_tensor("whxT", [D, D], bf16, kind="ExternalInput").ap()
    ct_d = nc.dram_tensor("ct", [D, D], bf16, kind="ExternalInput").ap()
    f_d = nc.dram_tensor("fmat", [L, D], bf16, kind="ExternalInput").ap()
    wu_d = nc.dram_tensor("wu", [128, KC], bf16, kind="ExternalInput").ap()
    vecs_d = nc.dram_tensor("vecs", [128, KC, 3], f32, kind="ExternalInput").ap()
    out_d = nc.dram_tensor("out", [D, BPC, L], bf16, kind="ExternalOutput").ap()
    upad_d = nc.dram_tensor("upad", [BPC * 2 * L], bf16).ap()  # internal scratch

    # x/out DRAM layout is [D, BPC, L] (host pre/post-transposes): each
    # partition's per-block run is NB*L contiguous elements -> 2KB DMA lines
    XSTR_D, XSTR_B = BPC * L, L

    with TileContext(nc) as tc:
        with (
            tc.tile_pool(name="const", bufs=1) as const,
            tc.tile_pool(name="xpool", bufs=4) as xpool,
            tc.tile_pool(name="hpool", bufs=12) as hpool,
            tc.tile_pool(name="tpool", bufs=2) as tpool,
            tc.tile_pool(name="opool", bufs=6) as opool,
            tc.tile_pool(name="upool", bufs=2) as upool,
            tc.tile_pool(name="pu", bufs=2, space="PSUM") as pu,
            tc.tile_pool(name="p3", bufs=3, space="PSUM") as p3,
            tc.tile_pool(name="p4", bufs=3, space="PSUM") as p4,
        ):
            # ---- constant tiles (DMA'd directly, already bf16 on host) ----
            whxT_r = const.tile([128, KC, D], bf16)    # [d' part | i_chunk | d]
            ct_r = const.tile([128, KC, D], bf16)      # [dh part | i_chunk | o]
            f_r = const.tile([128, D], bf16)           # [t' part | d]
            vecs_sb = const.tile([128, KC, 3], f32)    # b_h, bias', b_u
            wu_r = const.tile([128, KC], bf16)

            # minimal prologue: only what u(cb0) needs, so the PE starts
            # within a few microseconds of kernel entry. Param DRAM layout is
            # partition-major so each partition's free-dim run is contiguous
            # (per-element descriptor lines here cost ~10ns each and can add
            # ~8us to the critical path otherwise).
            nc.scalar.dma_start(out=wu_r[:], in_=wu_d)
            nc.scalar.dma_start(out=vecs_sb[:], in_=vecs_d)
            # zero the upad scratch (pad halves stay zero forever)
            zt = const.tile([128, 2 * BPC], bf16)
            nc.vector.memset(zt[:], 0.0)
            nc.sync.dma_start(
                out=bass.AP(tensor=upad_d.tensor, offset=0,
                            ap=[[1, BPC * 2 * L]]),
                in_=zt[:],
            )

            # whx is spread across all three queues (needed by ~12us for
            # step3(0)); ct rides the slow gpsimd queue — step4(0) needs it
            # ~8us later and nothing latency-critical shares that queue.
            # Few, LARGE DMAs everywhere: the tile framework has ~19 DMA
            # semaphore slots, and exceeding them serializes the warmup on
            # recycled slots.
            def stage_whx():
                for i in range(KC):
                    eng = (nc.sync, nc.scalar, nc.gpsimd)[i % 3]
                    eng.dma_start(
                        out=whxT_r[:, i, :],
                        in_=bass.AP(tensor=whxT_d.tensor, offset=i * 128 * D,
                                    ap=[[D, 128], [1, D]]),
                    )

            def stage_f():
                nc.sync.dma_start(out=f_r[:], in_=f_d)

            def stage_ct():
                for half in range(2):
                    nc.gpsimd.dma_start(
                        out=ct_r[:, half * 3:half * 3 + 3, :],
                        in_=bass.AP(tensor=ct_d.tensor,
                                    offset=half * 3 * 128 * D,
                                    ap=[[D, 128], [128 * D, 3], [1, D]]),
                    )

            def load_x(b0, nb, eng):
                """One DMA for a whole x block (all KC chunks, nb batches)."""
                xt = xpool.tile([128, KC, NCOL], bf16, tag="xt")
                eng.dma_start(
                    out=xt[:, :, :nb * L],
                    in_=bass.AP(
                        tensor=x_d.tensor,
                        offset=b0 * XSTR_B,
                        ap=[[XSTR_D, 128], [128 * XSTR_D, KC], [1, nb * L]],
                    ),
                )
                return xt

            def compute_u(b0, nb, xt):
                """u = relu(W_u @ x + b_u) -> upad scratch -> Toeplitz tile."""
                nc_ = nb * L
                psu = pu.tile([1, NCOL], f32, tag="pu")
                for i in range(KC):
                    nc.tensor.matmul(psu[:, :nc_], wu_r[:, i:i + 1],
                                     xt[:, i, :nc_],
                                     start=(i == 0), stop=(i == KC - 1))
                u_sb = upool.tile([1, NCOL], bf16, tag="u")
                # u = relu(psu * 1 + b_u)
                nc.scalar.activation(u_sb[:, :nc_], psu[:, :nc_],
                                     mybir.ActivationFunctionType.Relu,
                                     bias=vecs_sb[0:1, 0, 2:3])
                nc.scalar.dma_start(
                    out=bass.AP(tensor=upad_d.tensor,
                                offset=b0 * 2 * L + L,
                                ap=[[2 * L, nb], [1, L]]),
                    in_=u_sb[:, :nc_],
                )
                t_r = tpool.tile([128, NCOL], bf16, tag="tr")
                nc.scalar.dma_start(
                    out=t_r[:, :nc_],
                    in_=bass.AP(tensor=upad_d.tensor,
                                offset=b0 * 2 * L + 1,
                                ap=[[1, 128], [2 * L, nb], [1, L]]),
                )
                return t_r

            def step3(b0, nb, xt, t_r):
                nc_ = nb * L
                hs = []
                for j in range(KC):
                    ps3 = p3.tile([128, NCOL], f32, tag="ps3")
                    for i in range(KC):
                        nc.tensor.matmul(ps3[:, :nc_],
                                         whxT_r[:, i, j * 128:(j + 1) * 128],
                                         xt[:, i, :nc_], start=(i == 0), stop=False)
                    nc.tensor.matmul(ps3[:, :nc_], f_r[:, j * 128:(j + 1) * 128],
                                     t_r[:, :nc_], start=False, stop=True)
                    hj = hpool.tile([128, NCOL], bf16, tag="h")
                    nc.scalar.activation(hj[:, :nc_], ps3[:, :nc_],
                                         mybir.ActivationFunctionType.Relu,
                                         bias=vecs_sb[:, j, 0:1])
                    hs.append(hj)
                return hs

            def step4(b0, nb, hs):
                nc_ = nb * L
                for j in range(KC):
                    ps4 = p4.tile([128, NCOL], f32, tag="ps4")
                    for i in range(KC):
                        nc.tensor.matmul(ps4[:, :nc_],
                                         ct_r[:, i, j * 128:(j + 1) * 128],
                                         hs[i][:, :nc_],
                                         start=(i == 0), stop=(i == KC - 1))
                    oj = opool.tile([128, NCOL], bf16, tag="o")
                    nc.vector.tensor_scalar_add(oj[:, :nc_], ps4[:, :nc_],
                                                vecs_sb[:, j, 1:2])
                    oeng = nc.sync if j % 2 == 0 else nc.scalar
                    oeng.dma_start(
                        out=bass.AP(
                            tensor=out_d.tensor,
                            offset=b0 * XSTR_B + j * 128 * XSTR_D,
                            ap=[[XSTR_D, 128], [1, nb * L]],
                        ),
                        in_=oj[:, :nc_],
                    )

            # software pipeline: x blocks are prefetched two blocks ahead,
            # and the u-chain for block cb+1 runs BETWEEN step3(cb) and
            # step4(cb): its Toeplitz tile is needed ~9us later (F matmul of
            # step3(cb+1)), and its x block gets a full extra block-time to
            # arrive. Warmup DMAs are sequenced by need (x0 -> whx/f ->
            # ct/x1 -> x2) via tile_wait_until, since the scheduler's own
            # DMA model is too optimistic to order an HBM-saturated warmup.
            sizes = [NB] * NCB
            b0s = [sum(sizes[:k]) for k in range(len(sizes))]
            nblk = len(sizes)
            # block 0 arrives as two half-block DMAs on parallel queues
            with tc.high_priority():
                xt0 = xpool.tile([128, KC, NCOL], bf16, tag="xt")
                for h, eng in enumerate((nc.sync, nc.scalar)):
                    eng.dma_start(
                        out=xt0[:, :, h * (NCOL // 2):(h + 1) * (NCOL // 2)],
                        in_=bass.AP(
                            tensor=x_d.tensor,
                            offset=h * (NB // 2) * XSTR_B,
                            ap=[[XSTR_D, 128], [128 * XSTR_D, KC],
                                [1, NB * L // 2]],
                        ),
                    )
                xr = [xt0]
            with tc.tile_wait_until(0.0105):
                stage_whx()
                stage_f()
            t_cur = compute_u(b0s[0], sizes[0], xr[0])
            with tc.tile_wait_until(0.014):
                stage_ct()
                xr.append(load_x(b0s[1], sizes[1], nc.scalar))
            for cb in range(nblk):
                if cb + 2 < nblk:
                    with tc.tile_wait_until(0.019 + 0.005 * cb, enable=cb < 4):
                        xr.append(load_x(b0s[cb + 2], sizes[cb + 2],
                                         nc.sync if cb % 2 == 0 else nc.scalar))
                hs = step3(b0s[cb], sizes[cb], xr[cb], t_cur)
                t_cur = (compute_u(b0s[cb + 1], sizes[cb + 1], xr[cb + 1])
                         if cb + 1 < nblk else None)
                step4(b0s[cb], sizes[cb], hs)
                xr[cb] = None

    if not nc.is_finalized():
        nc.finalize()
    return nc


def _get_nc():
    global _NC_CACHE
    if _NC_CACHE is None:
        _NC_CACHE = _build_nc()
    return _NC_CACHE


def _ensure_ntff_hook():
    """Register the NTFF profile hook if the deployment lacks antenv.axon_hooks."""
    import sys
    import types
    try:
        from antenv.axon_hooks import get_axon_ntff_profile_hook  # noqa: F401
        return
    except ImportError:
        pass
    try:
        from trn_agent_boot.trn_boot import _ntff_profile_via_ctypes
        hook = _ntff_profile_via_ctypes("/opt/axon/libaxon_pjrt.so")
        mod = types.ModuleType("antenv.axon_hooks")
        mod.get_axon_ntff_profile_hook = lambda: hook
        mod.set_axon_ntff_profile_hook = lambda h: None
        import antenv
        sys.modules["antenv.axon_hooks"] = mod
        antenv.axon_hooks = mod
    except Exception:
        pass


def kernel(x, W_u, b_u, W_h, b_h, conv_w, conv_b, bn_gamma, bn_beta, bn_mean,
           bn_var):
    global LAST_EXEC_NS
    bf16 = ml_dtypes.bfloat16
    x = np.ascontiguousarray(np.asarray(x, dtype=np.float32)).astype(bf16)
    W_u = np.asarray(W_u, dtype=np.float64)
    b_u = np.asarray(b_u, dtype=np.float64)
    W_h = np.asarray(W_h, dtype=np.float64)
    b_h = np.asarray(b_h, dtype=np.float64)
    conv_w = np.asarray(conv_w, dtype=np.float64)
    conv_b = np.asarray(conv_b, dtype=np.float64)
    bn_gamma = np.asarray(bn_gamma, dtype=np.float64)
    bn_beta = np.asarray(bn_beta, dtype=np.float64)
    bn_mean = np.asarray(bn_mean, dtype=np.float64)
    bn_var = np.asarray(bn_var, dtype=np.float64)
    assert x.shape == (B, D, L)

    H = _impulse_response().astype(np.float64)  # [D, L]

    # host folds (O(params) only)
    F = (W_h[:, :D] @ H).T[::-1, :]                      # [L, D], row-flipped
    whxT = np.ascontiguousarray(W_h[:, D:].T)            # [D(d'), D(d)]
    inv = bn_gamma / np.sqrt(bn_var + BN_EPS)
    ct = np.ascontiguousarray((conv_w[:, :, 0] * inv[:, None]).T)  # [dh, o]
    bias2 = (conv_b - bn_mean) * inv + bn_beta
    # [128, KC, 3]: partition-major, contiguous free-dim run per partition
    vecs = np.stack([b_h, bias2, np.full(D, b_u[0])], axis=1)  # [D, 3]
    vecs = np.ascontiguousarray(vecs.reshape(KC, 128, 3).transpose(1, 0, 2))

    nc = _get_nc()
    shared = {
        "whxT": whxT.astype(np.float32).astype(bf16),
        "ct": ct.astype(np.float32).astype(bf16),
        "fmat": np.ascontiguousarray(F).astype(np.float32).astype(bf16),
        "wu": np.ascontiguousarray(
            W_u[0].astype(np.float32).astype(bf16).reshape(KC, 128).T),
        "vecs": vecs.astype(np.float32),
    }
    in_maps = []
    for c in range(NCORES):
        m = dict(shared)
        # [D, BPC, L] layout: 2KB contiguous DMA lines per partition
        m["x"] = np.ascontiguousarray(
            x[c * BPC:(c + 1) * BPC].transpose(1, 0, 2))
        in_maps.append(m)

    if TRACE:
        _ensure_ntff_hook()
    res = run_bass_kernel_spmd(nc, in_maps, list(range(NCORES)), trace=TRACE)
    LAST_EXEC_NS = res.exec_time_ns
    out = np.concatenate(
        [res.results[c]["out"].transpose(1, 0, 2) for c in range(NCORES)],
        axis=0)
    return out.astype(np.float32)


# revision 44
# speedup vs baseline: 1.0196x; 1.0196x over previous
"""LMU kernel for Trainium2, 8-core data-parallel.

Math (per batch b, with x[b] in [D, L] layout):
  u[b]    = relu(W_u @ x[b] + b_u)                              [1, L]
  m[b]    = H @ Toep(u[b])        (causal conv via Toeplitz)    [D, L]
  h[b]    = relu(W_h[:, :D] @ m[b] + W_h[:, D:] @ x[b] + b_h)   [D, L]
  y[b]    = BN(conv_w @ h[b] + conv_b)                          [D, L]

Device-side folds (host precomputes, O(params) only):
  F      = (W_h[:, :D] @ H).T, row-flipped  -> single K=128 contraction
           against the (flipped) Toeplitz of u
  C'     = (inv * conv_w).T, bias' = (conv_b - mean) * inv + beta   (BN fold)

All matmul operands are bf16 (host-cast), so LDWEIGHTS hides fully under
the 1 col/cycle stream and no on-device casts are needed anywhere.
Batch dim sharded 8 ways; params replicated.
"""

import os
import numpy as np
import ml_dtypes

import concourse.bass as bass
import concourse.mybir as mybir
from concourse import bacc
from concourse.tile import TileContext
from concourse.bass_utils import run_bass_kernel_spmd

B, D, L = 256, 768, 128
NCORES = 8
BPC = B // NCORES          # batches per core
NB = 4                     # batches per column block
NCB = BPC // NB            # column blocks per core
NCOL = NB * L              # 512 columns per block
KC = D // 128              # 6 chunks of 128 over the D dim
THETA = 128.0
BN_EPS = 1e-5

TRACE = False
LAST_EXEC_NS = None

_H_CACHE = None
_NC_CACHE = None


def _impulse_response():
    """Replicates the reference's H = impulse response [D, L], on CPU."""
    global _H_CACHE
    if _H_CACHE is not None:
        return _H_CACHE
    import jax
    import jax.numpy as jnp
    from jax.scipy.linalg import expm

    cpu = jax.devices("cpu")[0]
    with jax.default_device(cpu):
        Q = np.arange(D, dtype=np.float32)
        R = ((2.0 * Q + 1.0) / THETA)[:, None]
        i, j = np.meshgrid(Q, Q, indexing="ij")
        A = (np.where(i < j, -1.0, (-1.0) ** (i - j + 1)).astype(np.float32)) * R
        Bm = (((-1.0) ** Q)[:, None]).astype(np.float32) * R
        Maug = np.zeros((D + 1, D + 1), dtype=np.float32)
        Maug[:D, :D] = A
        Maug[:D, D:] = Bm
        E = expm(jnp.asarray(Maug))
        Ad = E[:D, :D]
        Bd = E[:D, D:]

        def step(Apow, _):
            return Ad @ Apow, (Apow @ Bd)[:, 0]

        _, H = jax.lax.scan(step, jnp.eye(D, dtype=jnp.float32), None, length=L)
        _H_CACHE = np.asarray(H).T.astype(np.float32)  # [D, L]
    return _H_CACHE


def _build_nc():
    """Builds the (static) 8-core SPMD Bass program."""
    f32 = mybir.dt.float32
    bf16 = mybir.dt.bfloat16
    nc = bacc.Bacc("TRN2", target_bir_lowering=False, debug=False, num_devices=NCORES)

    x_d = nc.dram_tensor("x", [D, BPC, L], bf16, kind="ExternalInput").ap()
    whxT_d = nc.dram_tensor("whxT", [D, D], bf16, kind="ExternalInput").ap()
    ct_d = nc.dram_tensor("ct", [D, D], bf16, kind="ExternalInput").ap()
    f_d = nc.dram_tensor("fmat", [L, D], bf16, kind="ExternalInput").ap()
    wu_d = nc.dram_tensor("wu", [128, KC], bf16, kind="ExternalInput").ap()
    vecs_d = nc.dram_tensor("vecs", [128, KC, 3], f32, kind="ExternalInput").ap()
    out_d = nc.dram_tensor("out", [D, BPC, L], bf16, kind="ExternalOutput").ap()
    upad_d = nc.dram_tensor("upad", [BPC * 2 * L], bf16).ap()  # internal scratch

    # x/out DRAM layout is [D, BPC, L] (host pre/post-transposes): each
    # partition's per-block run is NB*L contiguous elements -> 2KB DMA lines
    XSTR_D, XSTR_B = BPC * L, L

    with TileContext(nc) as tc:
        with (
            tc.tile_pool(name="const", bufs=1) as const,
            tc.tile_pool(name="xpool", bufs=4) as xpool,
            tc.tile_pool(name="hpool", bufs=12) as hpool,
            tc.tile_pool(name="tpool", bufs=2) as tpool,
            tc.tile_pool(name="opool", bufs=6) as opool,
            tc.tile_pool(name="upool", bufs=2) as upool,
            tc.tile_pool(name="pu", bufs=1, space="PSUM") as pu,
            tc.tile_pool(name="p3", bufs=4, space="PSUM") as p3,
            tc.tile_pool(name="p4", bufs=3, space="PSUM") as p4,
        ):
            # ---- constant tiles (DMA'd directly, already bf16 on host) ----
            whxT_r = const.tile([128, KC, D], bf16)    # [d' part | i_chunk | d]
            ct_r = const.tile([128, KC, D], bf16)      # [dh part | i_chunk | o]
            f_r = const.tile([128, D], bf16)           # [t' part | d]
            vecs_sb = const.tile([128, KC, 3], f32)    # b_h, bias', b_u
            wu_r = const.tile([128, KC], bf16)

            # minimal prologue: only what u(cb0) needs, so the PE starts
            # within a few microseconds of kernel entry. Param DRAM layout is
            # partition-major so each partition's free-dim run is contiguous
            # (per-element descriptor lines here cost ~10ns each and can add
            # ~8us to the critical path otherwise).
            nc.scalar.dma_start(out=wu_r[:], in_=wu_d)
            nc.scalar.dma_start(out=vecs_sb[:], in_=vecs_d)
            # zero the upad scratch (pad halves stay zero forever)
            zt = const.tile([128, 2 * BPC], bf16)
            nc.vector.memset(zt[:], 0.0)
            nc.sync.dma_start(
                out=bass.AP(tensor=upad_d.tensor, offset=0,
                            ap=[[1, BPC * 2 * L]]),
                in_=zt[:],
            )

            # whx is spread across all three queues (needed by ~12us for
            # step3(0)); ct rides the slow gpsimd queue — step4(0) needs it
            # ~8us later and nothing latency-critical shares that queue.
            # Few, LARGE DMAs everywhere: the tile framework has ~19 DMA
            # semaphore slots, and exceeding them serializes the warmup on
            # recycled slots.
            def stage_whx():
                for i in range(KC):
                    eng = (nc.sync, nc.scalar, nc.gpsimd)[i % 3]
                    eng.dma_start(
                        out=whxT_r[:, i, :],
                        in_=bass.AP(tensor=whxT_d.tensor, offset=i * 128 * D,
                                    ap=[[D, 128], [1, D]]),
                    )

            def stage_f():
                nc.sync.dma_start(out=f_r[:], in_=f_d)

            def stage_ct():
                for half in range(2):
                    nc.gpsimd.dma_start(
                        out=ct_r[:, half * 3:half * 3 + 3, :],
                        in_=bass.AP(tensor=ct_d.tensor,
                                    offset=half * 3 * 128 * D,
                                    ap=[[D, 128], [128 * D, 3], [1, D]]),
                    )

            def load_x(b0, nb, eng):
                """One DMA for a whole x block (all KC chunks, nb batches)."""
                xt = xpool.tile([128, KC, NCOL], bf16, tag="xt")
                eng.dma_start(
                    out=xt[:, :, :nb * L],
                    in_=bass.AP(
                        tensor=x_d.tensor,
                        offset=b0 * XSTR_B,
                        ap=[[XSTR_D, 128], [128 * XSTR_D, KC], [1, nb * L]],
                    ),
                )
                return xt

            def compute_u(b0, nb, xt):
                """u = relu(W_u @ x + b_u) -> upad scratch -> Toeplitz tile."""
                nc_ = nb * L
                psu = pu.tile([1, NCOL], f32, tag="pu")
                for i in range(KC):
                    nc.tensor.matmul(psu[:, :nc_], wu_r[:, i:i + 1],
                                     xt[:, i, :nc_],
                                     start=(i == 0), stop=(i == KC - 1))
                u_sb = upool.tile([1, NCOL], bf16, tag="u")
                # u = relu(psu * 1 + b_u)
                nc.scalar.activation(u_sb[:, :nc_], psu[:, :nc_],
                                     mybir.ActivationFunctionType.Relu,
                                     bias=vecs_sb[0:1, 0, 2:3])
                nc.scalar.dma_start(
                    out=bass.AP(tensor=upad_d.tensor,
                                offset=b0 * 2 * L + L,
                                ap=[[2 * L, nb], [1, L]]),
                    in_=u_sb[:, :nc_],
                )
                t_r = tpool.tile([128, NCOL], bf16, tag="tr")
                nc.scalar.dma_start(
                    out=t_r[:, :nc_],
                    in_=bass.AP(tensor=upad_d.tensor,
                                offset=b0 * 2 * L + 1,
                                ap=[[1, 128], [2 * L, nb], [1, L]]),
                )
                return t_r

            def step3(b0, nb, xt, t_r):
                # i-outer over half the j-chunks at a time: each arriving whx
                # chunk unlocks 3 matmuls (instead of 1), so block 0 streams
                # while whx staging trickles in, and the Toeplitz tile is not
                # needed until the F matmuls at the end of each half.
                nc_ = nb * L
                hs = []
                for half in range(2):
                    js = list(range(half * 3, half * 3 + 3))
                    tiles = [p3.tile([128, NCOL], f32, tag="ps3",
                                     name=f"ps3_{half}_{k}") for k in range(3)]
                    for i in range(KC):
                        for k, j in enumerate(js):
                            nc.tensor.matmul(tiles[k][:, :nc_],
                                             whxT_r[:, i, j * 128:(j + 1) * 128],
                                             xt[:, i, :nc_],
                                             start=(i == 0), stop=False)
                    for k, j in enumerate(js):
                        nc.tensor.matmul(tiles[k][:, :nc_],
                                         f_r[:, j * 128:(j + 1) * 128],
                                         t_r[:, :nc_], start=False, stop=True)
                        hj = hpool.tile([128, NCOL], bf16, tag="h")
                        nc.scalar.activation(hj[:, :nc_], tiles[k][:, :nc_],
                                             mybir.ActivationFunctionType.Relu,
                                             bias=vecs_sb[:, j, 0:1])
                        hs.append(hj)
                return hs

            def step4(b0, nb, hs):
                nc_ = nb * L
                for j in range(KC):
                    ps4 = p4.tile([128, NCOL], f32, tag="ps4")
                    for i in range(KC):
                        nc.tensor.matmul(ps4[:, :nc_],
                                         ct_r[:, i, j * 128:(j + 1) * 128],
                                         hs[i][:, :nc_],
                                         start=(i == 0), stop=(i == KC - 1))
                    oj = opool.tile([128, NCOL], bf16, tag="o")
                    nc.vector.tensor_scalar_add(oj[:, :nc_], ps4[:, :nc_],
                                                vecs_sb[:, j, 1:2])
                    oeng = nc.sync if j % 2 == 0 else nc.scalar
                    oeng.dma_start(
                        out=bass.AP(
                            tensor=out_d.tensor,
                            offset=b0 * XSTR_B + j * 128 * XSTR_D,
                            ap=[[XSTR_D, 128], [1, nb * L]],
                        ),
                        in_=oj[:, :nc_],
                    )

            # software pipeline: x blocks are prefetched two blocks ahead,
            # and the u-chain for block cb+1 runs BETWEEN step3(cb) and
            # step4(cb): its Toeplitz tile is needed ~9us later (F matmul of
            # step3(cb+1)), and its x block gets a full extra block-time to
            # arrive. Warmup DMAs are sequenced by need (x0 -> whx/f ->
            # ct/x1 -> x2) via tile_wait_until, since the scheduler's own
            # DMA model is too optimistic to order an HBM-saturated warmup.
            sizes = [NB] * NCB
            b0s = [sum(sizes[:k]) for k in range(len(sizes))]
            nblk = len(sizes)
            # block 0 arrives as two half-block DMAs on parallel queues
            with tc.high_priority():
                xt0 = xpool.tile([128, KC, NCOL], bf16, tag="xt")
                for h, eng in enumerate((nc.sync, nc.scalar)):
                    eng.dma_start(
                        out=xt0[:, :, h * (NCOL // 2):(h + 1) * (NCOL // 2)],
                        in_=bass.AP(
                            tensor=x_d.tensor,
                            offset=h * (NB // 2) * XSTR_B,
                            ap=[[XSTR_D, 128], [128 * XSTR_D, KC],
                                [1, NB * L // 2]],
                        ),
                    )
                xr = [xt0]
            with tc.tile_wait_until(0.002):
                stage_whx()
                stage_f()
            t_cur = compute_u(b0s[0], sizes[0], xr[0])
            with tc.tile_wait_until(0.005):
                stage_ct()
                xr.append(load_x(b0s[1], sizes[1], nc.scalar))
            for cb in range(nblk):
                if cb + 2 < nblk:
                    with tc.tile_wait_until(0.008 + 0.004 * cb, enable=cb < 4):
                        xr.append(load_x(b0s[cb + 2], sizes[cb + 2],
                                         nc.sync if cb % 2 == 0 else nc.scalar))
                hs = step3(b0s[cb], sizes[cb], xr[cb], t_cur)
                t_cur = (compute_u(b0s[cb + 1], sizes[cb + 1], xr[cb + 1])
                         if cb + 1 < nblk else None)
                step4(b0s[cb], sizes[cb], hs)
                xr[cb] = None

    if not nc.is_finalized():
        nc.finalize()
    return nc


def _get_nc():
    global _NC_CACHE
    if _NC_CACHE is None:
        _NC_CACHE = _build_nc()
    return _NC_CACHE


def _ensure_ntff_hook():
    """Register the NTFF profile hook if the deployment lacks antenv.axon_hooks."""
    import sys
    import types
    try:
        from antenv.axon_hooks import get_axon_ntff_profile_hook  # noqa: F401
        return
    except ImportError:
        pass
    try:
        from trn_agent_boot.trn_boot import _ntff_profile_via_ctypes
        hook = _ntff_profile_via_ctypes("/opt/axon/libaxon_pjrt.so")
        mod = types.ModuleType("antenv.axon_hooks")
        mod.get_axon_ntff_profile_hook = lambda: hook
        mod.set_axon_ntff_profile_hook = lambda h: None
        import antenv
        sys.modules["antenv.axon_hooks"] = mod
        antenv.axon_hooks = mod
    except Exception:
        pass


def kernel(x, W_u, b_u, W_h, b_h, conv_w, conv_b, bn_gamma, bn_beta, bn_mean,
           bn_var):
    global LAST_EXEC_NS
    bf16 = ml_dtypes.bfloat16
    x = np.ascontiguousarray(np.asarray(x, dtype=np.float32)).astype(bf16)
    W_u = np.asarray(W_u, dtype=np.float64)
    b_u = np.asarray(b_u, dtype=np.float64)
    W_h = np.asarray(W_h, dtype=np.float64)
    b_h = np.asarray(b_h, dtype=np.float64)
    conv_w = np.asarray(conv_w, dtype=np.float64)
    conv_b = np.asarray(conv_b, dtype=np.float64)
    bn_gamma = np.asarray(bn_gamma, dtype=np.float64)
    bn_beta = np.asarray(bn_beta, dtype=np.float64)
    bn_mean = np.asarray(bn_mean, dtype=np.float64)
    bn_var = np.asarray(bn_var, dtype=np.float64)
    assert x.shape == (B, D, L)

    H = _impulse_response().astype(np.float64)  # [D, L]

    # host folds (O(params) only)
    F = (W_h[:, :D] @ H).T[::-1, :]                      # [L, D], row-flipped
    whxT = np.ascontiguousarray(W_h[:, D:].T)            # [D(d'), D(d)]
    inv = bn_gamma / np.sqrt(bn_var + BN_EPS)
    ct = np.ascontiguousarray((conv_w[:, :, 0] * inv[:, None]).T)  # [dh, o]
    bias2 = (conv_b - bn_mean) * inv + bn_beta
    # [128, KC, 3]: partition-major, contiguous free-dim run per partition
    vecs = np.stack([b_h, bias2, np.full(D, b_u[0])], axis=1)  # [D, 3]
    vecs = np.ascontiguousarray(vecs.reshape(KC, 128, 3).transpose(1, 0, 2))

    nc = _get_nc()
    shared = {
        "whxT": whxT.astype(np.float32).astype(bf16),
        "ct": ct.astype(np.float32).astype(bf16),
        "fmat": np.ascontiguousarray(F).astype(np.float32).astype(bf16),
        "wu": np.ascontiguousarray(
            W_u[0].astype(np.float32).astype(bf16).reshape(KC, 128).T),
        "vecs": vecs.astype(np.float32),
    }
    in_maps = []
    for c in range(NCORES):
        m = dict(shared)
        # [D, BPC, L] layout: 2KB contiguous DMA lines per partition
        m["x"] = np.ascontiguousarray(
            x[c * BPC:(c + 1) * BPC].transpose(1, 0, 2))
        in_maps.append(m)

    if TRACE:
        _ensure_ntff_hook()
    res = run_bass_kernel_spmd(nc, in_maps, list(range(NCORES)), trace=TRACE)
    LAST_EXEC_NS = res.exec_time_ns
    out = np.concatenate(
        [res.results[c]["out"].transpose(1, 0, 2) for c in range(NCORES)],
        axis=0)
    return out.astype(np.float32)


# revision 46
# speedup vs baseline: 1.0225x; 1.0028x over previous
"""LMU kernel for Trainium2, 8-core data-parallel.

Math (per batch b, with x[b] in [D, L] layout):
  u[b]    = relu(W_u @ x[b] + b_u)                              [1, L]
  m[b]    = H @ Toep(u[b])        (causal conv via Toeplitz)    [D, L]
  h[b]    = relu(W_h[:, :D] @ m[b] + W_h[:, D:] @ x[b] + b_h)   [D, L]
  y[b]    = BN(conv_w @ h[b] + conv_b)                          [D, L]

Device-side folds (host precomputes, O(params) only):
  F      = (W_h[:, :D] @ H).T, row-flipped  -> single K=128 contraction
           against the (flipped) Toeplitz of u
  C'     = (inv * conv_w).T, bias' = (conv_b - mean) * inv + beta   (BN fold)

All matmul operands are bf16 (host-cast), so LDWEIGHTS hides fully under
the 1 col/cycle stream and no on-device casts are needed anywhere.
Batch dim sharded 8 ways; params replicated.
"""

import os
import numpy as np
import ml_dtypes

import concourse.bass as bass
import concourse.mybir as mybir
from concourse import bacc
from concourse.tile import TileContext
from concourse.bass_utils import run_bass_kernel_spmd

B, D, L = 256, 768, 128
NCORES = 8
BPC = B // NCORES          # batches per core
NB = 4                     # batches per column block
NCB = BPC // NB            # column blocks per core
NCOL = NB * L              # 512 columns per block
KC = D // 128              # 6 chunks of 128 over the D dim
THETA = 128.0
BN_EPS = 1e-5

TRACE = False
LAST_EXEC_NS = None

_H_CACHE = None
_NC_CACHE = None


def _impulse_response():
    """Replicates the reference's H = impulse response [D, L], on CPU."""
    global _H_CACHE
    if _H_CACHE is not None:
        return _H_CACHE
    import jax
    import jax.numpy as jnp
    from jax.scipy.linalg import expm

    cpu = jax.devices("cpu")[0]
    with jax.default_device(cpu):
        Q = np.arange(D, dtype=np.float32)
        R = ((2.0 * Q + 1.0) / THETA)[:, None]
        i, j = np.meshgrid(Q, Q, indexing="ij")
        A = (np.where(i < j, -1.0, (-1.0) ** (i - j + 1)).astype(np.float32)) * R
        Bm = (((-1.0) ** Q)[:, None]).astype(np.float32) * R
        Maug = np.zeros((D + 1, D + 1), dtype=np.float32)
        Maug[:D, :D] = A
        Maug[:D, D:] = Bm
        E = expm(jnp.asarray(Maug))
        Ad = E[:D, :D]
        Bd = E[:D, D:]

        def step(Apow, _):
            return Ad @ Apow, (Apow @ Bd)[:, 0]

        _, H = jax.lax.scan(step, jnp.eye(D, dtype=jnp.float32), None, length=L)
        _H_CACHE = np.asarray(H).T.astype(np.float32)  # [D, L]
    return _H_CACHE


def _build_nc():
    """Builds the (static) 8-core SPMD Bass program."""
    f32 = mybir.dt.float32
    bf16 = mybir.dt.bfloat16
    nc = bacc.Bacc("TRN2", target_bir_lowering=False, debug=False, num_devices=NCORES)

    x_d = nc.dram_tensor("x", [D, BPC, L], bf16, kind="ExternalInput").ap()
    whxT_d = nc.dram_tensor("whxT", [D, D], bf16, kind="ExternalInput").ap()
    ct_d = nc.dram_tensor("ct", [D, D], bf16, kind="ExternalInput").ap()
    f_d = nc.dram_tensor("fmat", [L, D], bf16, kind="ExternalInput").ap()
    wu_d = nc.dram_tensor("wu", [128, KC], bf16, kind="ExternalInput").ap()
    vecs_d = nc.dram_tensor("vecs", [128, KC, 3], f32, kind="ExternalInput").ap()
    out_d = nc.dram_tensor("out", [D, BPC, L], bf16, kind="ExternalOutput").ap()
    upad_d = nc.dram_tensor("upad", [BPC * 2 * L], bf16).ap()  # internal scratch

    # x/out DRAM layout is [D, BPC, L] (host pre/post-transposes): each
    # partition's per-block run is NB*L contiguous elements -> 2KB DMA lines
    XSTR_D, XSTR_B = BPC * L, L

    with TileContext(nc) as tc:
        with (
            tc.tile_pool(name="const", bufs=1) as const,
            tc.tile_pool(name="xpool", bufs=4) as xpool,
            tc.tile_pool(name="hpool", bufs=12) as hpool,
            tc.tile_pool(name="tpool", bufs=2) as tpool,
            tc.tile_pool(name="opool", bufs=6) as opool,
            tc.tile_pool(name="upool", bufs=2) as upool,
            tc.tile_pool(name="pu", bufs=1, space="PSUM") as pu,
            tc.tile_pool(name="p3", bufs=4, space="PSUM") as p3,
            tc.tile_pool(name="p4", bufs=3, space="PSUM") as p4,
        ):
            # ---- constant tiles (DMA'd directly, already bf16 on host) ----
            whxT_r = const.tile([128, KC, D], bf16)    # [d' part | i_chunk | d]
            ct_r = const.tile([128, KC, D], bf16)      # [dh part | i_chunk | o]
            f_r = const.tile([128, D], bf16)           # [t' part | d]
            vecs_sb = const.tile([128, KC, 3], f32)    # b_h, bias', b_u
            wu_r = const.tile([128, KC], bf16)

            # minimal prologue: only what u(cb0) needs, so the PE starts
            # within a few microseconds of kernel entry. Param DRAM layout is
            # partition-major so each partition's free-dim run is contiguous
            # (per-element descriptor lines here cost ~10ns each and can add
            # ~8us to the critical path otherwise).
            nc.scalar.dma_start(out=wu_r[:], in_=wu_d)
            nc.scalar.dma_start(out=vecs_sb[:], in_=vecs_d)
            # zero the upad scratch (pad halves stay zero forever)
            zt = const.tile([128, 2 * BPC], bf16)
            nc.vector.memset(zt[:], 0.0)
            nc.sync.dma_start(
                out=bass.AP(tensor=upad_d.tensor, offset=0,
                            ap=[[1, BPC * 2 * L]]),
                in_=zt[:],
            )

            # whx is spread across all three queues (needed by ~12us for
            # step3(0)); ct rides the slow gpsimd queue — step4(0) needs it
            # ~8us later and nothing latency-critical shares that queue.
            # Few, LARGE DMAs everywhere: the tile framework has ~19 DMA
            # semaphore slots, and exceeding them serializes the warmup on
            # recycled slots.
            def stage_whx():
                for i in range(KC):
                    eng = (nc.sync, nc.scalar, nc.gpsimd)[i % 3]
                    eng.dma_start(
                        out=whxT_r[:, i, :],
                        in_=bass.AP(tensor=whxT_d.tensor, offset=i * 128 * D,
                                    ap=[[D, 128], [1, D]]),
                    )

            def stage_f():
                nc.sync.dma_start(out=f_r[:], in_=f_d)

            def stage_ct():
                for half in range(2):
                    nc.gpsimd.dma_start(
                        out=ct_r[:, half * 3:half * 3 + 3, :],
                        in_=bass.AP(tensor=ct_d.tensor,
                                    offset=half * 3 * 128 * D,
                                    ap=[[D, 128], [128 * D, 3], [1, D]]),
                    )

            def load_x(b0, nb, eng):
                """One DMA for a whole x block (all KC chunks, nb batches)."""
                xt = xpool.tile([128, KC, NCOL], bf16, tag="xt")
                eng.dma_start(
                    out=xt[:, :, :nb * L],
                    in_=bass.AP(
                        tensor=x_d.tensor,
                        offset=b0 * XSTR_B,
                        ap=[[XSTR_D, 128], [128 * XSTR_D, KC], [1, nb * L]],
                    ),
                )
                return xt

            def compute_u(b0, nb, xt):
                """u = relu(W_u @ x + b_u) -> upad scratch -> Toeplitz tile."""
                nc_ = nb * L
                psu = pu.tile([1, NCOL], f32, tag="pu")
                for i in range(KC):
                    nc.tensor.matmul(psu[:, :nc_], wu_r[:, i:i + 1],
                                     xt[:, i, :nc_],
                                     start=(i == 0), stop=(i == KC - 1))
                u_sb = upool.tile([1, NCOL], bf16, tag="u")
                # u = relu(psu * 1 + b_u)
                nc.scalar.activation(u_sb[:, :nc_], psu[:, :nc_],
                                     mybir.ActivationFunctionType.Relu,
                                     bias=vecs_sb[0:1, 0, 2:3])
                nc.scalar.dma_start(
                    out=bass.AP(tensor=upad_d.tensor,
                                offset=b0 * 2 * L + L,
                                ap=[[2 * L, nb], [1, L]]),
                    in_=u_sb[:, :nc_],
                )
                t_r = tpool.tile([128, NCOL], bf16, tag="tr")
                nc.scalar.dma_start(
                    out=t_r[:, :nc_],
                    in_=bass.AP(tensor=upad_d.tensor,
                                offset=b0 * 2 * L + 1,
                                ap=[[1, 128], [2 * L, nb], [1, L]]),
                )
                return t_r

            def step3(b0, nb, xt, t_r):
                # i-outer over half the j-chunks at a time: each arriving whx
                # chunk unlocks 3 matmuls (instead of 1), so block 0 streams
                # while whx staging trickles in, and the Toeplitz tile is not
                # needed until the F matmuls at the end of each half.
                nc_ = nb * L
                hs = []
                for half in range(2):
                    js = list(range(half * 3, half * 3 + 3))
                    tiles = [p3.tile([128, NCOL], f32, tag="ps3",
                                     name=f"ps3_{half}_{k}") for k in range(3)]
                    for i in range(KC):
                        for k, j in enumerate(js):
                            nc.tensor.matmul(tiles[k][:, :nc_],
                                             whxT_r[:, i, j * 128:(j + 1) * 128],
                                             xt[:, i, :nc_],
                                             start=(i == 0), stop=False)
                    for k, j in enumerate(js):
                        nc.tensor.matmul(tiles[k][:, :nc_],
                                         f_r[:, j * 128:(j + 1) * 128],
                                         t_r[:, :nc_], start=False, stop=True)
                        hj = hpool.tile([128, NCOL], bf16, tag="h")
                        nc.scalar.activation(hj[:, :nc_], tiles[k][:, :nc_],
                                             mybir.ActivationFunctionType.Relu,
                                             bias=vecs_sb[:, j, 0:1])
                        hs.append(hj)
                return hs

            def step4(b0, nb, hs, last=False):
                nc_ = nb * L
                for j in range(KC):
                    ps4 = p4.tile([128, NCOL], f32, tag="ps4")
                    for i in range(KC):
                        nc.tensor.matmul(ps4[:, :nc_],
                                         ct_r[:, i, j * 128:(j + 1) * 128],
                                         hs[i][:, :nc_],
                                         start=(i == 0), stop=(i == KC - 1))
                    oj = opool.tile([128, NCOL], bf16, tag="o")
                    # the very last chunk drains as two halves on both hw
                    # queues so the final DMA (whose completion ends the
                    # measured window) is as small and early as possible
                    splits = 2 if (last and j == KC - 1) else 1
                    for s in range(splits):
                        w = nc_ // splits
                        sl = slice(s * w, (s + 1) * w)
                        nc.vector.tensor_scalar_add(oj[:, sl], ps4[:, sl],
                                                    vecs_sb[:, j, 1:2])
                        oeng = (nc.sync, nc.scalar)[(j + s) % 2]
                        oeng.dma_start(
                            out=bass.AP(
                                tensor=out_d.tensor,
                                offset=b0 * XSTR_B + j * 128 * XSTR_D + s * w,
                                ap=[[XSTR_D, 128], [1, w]],
                            ),
                            in_=oj[:, sl],
                        )

            # software pipeline: x blocks are prefetched two blocks ahead,
            # and the u-chain for block cb+1 runs BETWEEN step3(cb) and
            # step4(cb): its Toeplitz tile is needed ~9us later (F matmul of
            # step3(cb+1)), and its x block gets a full extra block-time to
            # arrive. Warmup DMAs are sequenced by need (x0 -> whx/f ->
            # ct/x1 -> x2) via tile_wait_until, since the scheduler's own
            # DMA model is too optimistic to order an HBM-saturated warmup.
            sizes = [NB] * NCB
            b0s = [sum(sizes[:k]) for k in range(len(sizes))]
            nblk = len(sizes)
            # block 0 arrives as two half-block DMAs on parallel queues
            with tc.high_priority():
                xt0 = xpool.tile([128, KC, NCOL], bf16, tag="xt")
                for h, eng in enumerate((nc.sync, nc.scalar)):
                    eng.dma_start(
                        out=xt0[:, :, h * (NCOL // 2):(h + 1) * (NCOL // 2)],
                        in_=bass.AP(
                            tensor=x_d.tensor,
                            offset=h * (NB // 2) * XSTR_B,
                            ap=[[XSTR_D, 128], [128 * XSTR_D, KC],
                                [1, NB * L // 2]],
                        ),
                    )
                xr = [xt0]
            with tc.tile_wait_until(0.002):
                stage_whx()
                stage_f()
            t_cur = compute_u(b0s[0], sizes[0], xr[0])
            with tc.tile_wait_until(0.005):
                stage_ct()
                xr.append(load_x(b0s[1], sizes[1], nc.scalar))
            for cb in range(nblk):
                if cb + 2 < nblk:
                    with tc.tile_wait_until(0.008 + 0.004 * cb, enable=cb < 4):
                        xr.append(load_x(b0s[cb + 2], sizes[cb + 2],
                                         nc.sync if cb % 2 == 0 else nc.scalar))
                hs = step3(b0s[cb], sizes[cb], xr[cb], t_cur)
                t_cur = (compute_u(b0s[cb + 1], sizes[cb + 1], xr[cb + 1])
                         if cb + 1 < nblk else None)
                step4(b0s[cb], sizes[cb], hs, last=(cb == nblk - 1))
                xr[cb] = None

    if not nc.is_finalized():
        nc.finalize()
    return nc


def _get_nc():
    global _NC_CACHE
    if _NC_CACHE is None:
        _NC_CACHE = _build_nc()
    return _NC_CACHE


def _ensure_ntff_hook():
    """Register the NTFF profile hook if the deployment lacks antenv.axon_hooks."""
    import sys
    import types
    try:
        from antenv.axon_hooks import get_axon_ntff_profile_hook  # noqa: F401
        return
    except ImportError:
        pass
    try:
        from trn_agent_boot.trn_boot import _ntff_profile_via_ctypes
        hook = _ntff_profile_via_ctypes("/opt/axon/libaxon_pjrt.so")
        mod = types.ModuleType("antenv.axon_hooks")
        mod.get_axon_ntff_profile_hook = lambda: hook
        mod.set_axon_ntff_profile_hook = lambda h: None
        import antenv
        sys.modules["antenv.axon_hooks"] = mod
        antenv.axon_hooks = mod
    except Exception:
        pass


def kernel(x, W_u, b_u, W_h, b_h, conv_w, conv_b, bn_gamma, bn_beta, bn_mean,
           bn_var):
    global LAST_EXEC_NS
    bf16 = ml_dtypes.bfloat16
    x = np.ascontiguousarray(np.asarray(x, dtype=np.float32)).astype(bf16)
    W_u = np.asarray(W_u, dtype=np.float64)
    b_u = np.asarray(b_u, dtype=np.float64)
    W_h = np.asarray(W_h, dtype=np.float64)
    b_h = np.asarray(b_h, dtype=np.float64)
    conv_w = np.asarray(conv_w, dtype=np.float64)
    conv_b = np.asarray(conv_b, dtype=np.float64)
    bn_gamma = np.asarray(bn_gamma, dtype=np.float64)
    bn_beta = np.asarray(bn_beta, dtype=np.float64)
    bn_mean = np.asarray(bn_mean, dtype=np.float64)
    bn_var = np.asarray(bn_var, dtype=np.float64)
    assert x.shape == (B, D, L)

    H = _impulse_response().astype(np.float64)  # [D, L]

    # host folds (O(params) only)
    F = (W_h[:, :D] @ H).T[::-1, :]                      # [L, D], row-flipped
    whxT = np.ascontiguousarray(W_h[:, D:].T)            # [D(d'), D(d)]
    inv = bn_gamma / np.sqrt(bn_var + BN_EPS)
    ct = np.ascontiguousarray((conv_w[:, :, 0] * inv[:, None]).T)  # [dh, o]
    bias2 = (conv_b - bn_mean) * inv + bn_beta
    # [128, KC, 3]: partition-major, contiguous free-dim run per partition
    vecs = np.stack([b_h, bias2, np.full(D, b_u[0])], axis=1)  # [D, 3]
    vecs = np.ascontiguousarray(vecs.reshape(KC, 128, 3).transpose(1, 0, 2))

    nc = _get_nc()
    shared = {
        "whxT": whxT.astype(np.float32).astype(bf16),
        "ct": ct.astype(np.float32).astype(bf16),
        "fmat": np.ascontiguousarray(F).astype(np.float32).astype(bf16),
        "wu": np.ascontiguousarray(
            W_u[0].astype(np.float32).astype(bf16).reshape(KC, 128).T),
        "vecs": vecs.astype(np.float32),
    }
    in_maps = []
    for c in range(NCORES):
        m = dict(shared)
        # [D, BPC, L] layout: 2KB contiguous DMA lines per partition
        m["x"] = np.ascontiguousarray(
            x[c * BPC:(c + 1) * BPC].transpose(1, 0, 2))
        in_maps.append(m)

    if TRACE:
        _ensure_ntff_hook()
    res = run_bass_kernel_spmd(nc, in_maps, list(range(NCORES)), trace=TRACE)
    LAST_EXEC_NS = res.exec_time_ns
    out = np.concatenate(
        [res.results[c]["out"].transpose(1, 0, 2) for c in range(NCORES)],
        axis=0)
    return out.astype(np.float32)
